# revision 15
# baseline (speedup 1.0000x reference)
"""Trainium2 Bass kernel for nn_HalfConv_876173328516 (GNN message passing).

Strategy
--------
Host: sort edges by e_idx_u; core k owns u rows [k*6250, (k+1)*6250), so the 8
cores are fully independent (no collectives). Per-edge inputs are expanded and
transposed on the host into a [128, NE] u||v feature stream plus a packed
[112, NE/4] e_vals stream per core (4 chunks side by side in PE row groups
0-15/32-47/64-79/96-111), with edges packed into 512-edge chunks that
(a) never split one u across chunks and (b) span < 64 u-slots.

Device (per core, per superblock of 4 chunks = 2048 edges):
  L1   z1[q] = W1uv.T @ x_q       (4 matmuls, one w1a weight-load context)
       z1[q] += W1e.T @ e_q       (4 K=16 matmuls packed in disjoint PE row
                                   groups -> run concurrently)
       h1 = relu(z1 + b1)         (ACT, -> SBUF bf16)
  L2   per 128-edge group: z2 = h1_g.T @ W2    (edge-major output)
       h2 = relu(z2 + b2)                      (DVE add + max)
  SUM  one-hot[e, slot] built once per superblock ([128, 1024] is_equal)
       pT[64 feats, 64 slots/chunk] += h2_g.T @ oh_g  (PSUM accumulate)
       flush pT -> xf[0:64, 256*m4:...]        (one DVE copy per superblock)
  f-MLP over all compact slot columns: xf = [aggT ; uT], two matmuls + relus.

Host: out[u] = out_T[:, col_of_slot[u]].T per core.
"""

import numpy as np

U, V, E = 50000, 50000, 800000
NCORES = 8
U_PER = U // NCORES          # 6250
CHUNK = 512                  # edges per chunk
GP = 128                     # edges per matmul group
GROUPS = CHUNK // GP         # 4
WSLOTS = 64                  # slot window per chunk
SB = 4                       # chunks per superblock


# ---------------------------------------------------------------- host side

def _preprocess(u, v, e_vals, e_idx_v, e_idx_u):
    u = np.ascontiguousarray(np.asarray(u, np.float32))
    v = np.ascontiguousarray(np.asarray(v, np.float32))
    e_vals = np.ascontiguousarray(np.asarray(e_vals, np.float32))
    e_idx_u = np.asarray(e_idx_u).astype(np.int64)
    e_idx_v = np.asarray(e_idx_v).astype(np.int64)

    perm = np.argsort(e_idx_u, kind="stable")
    su = e_idx_u[perm]
    sv = e_idx_v[perm]
    se = e_vals[perm]

    core_lo = np.searchsorted(su, np.arange(NCORES) * U_PER, side="left")
    core_hi = np.searchsorted(su, (np.arange(NCORES) + 1) * U_PER, side="left")

    cores = []
    for k in range(NCORES):
        lo, hi = int(core_lo[k]), int(core_hi[k])
        su_l = (su[lo:hi] - k * U_PER).astype(np.int64)
        n = hi - lo
        starts, bases = [], []
        i = 0
        while i < n:
            base = int(su_l[i])
            j = min(i + CHUNK, n)
            j = min(j, int(np.searchsorted(su_l, base + WSLOTS, side="left")))
            if j < n:
                # step back to a u-boundary so no u straddles chunks
                j2 = int(np.searchsorted(su_l, su_l[j - 1], side="left"))
                if j2 > i and su_l[j - 1] == su_l[j]:
                    j = j2
            assert j > i, "u degree >= CHUNK unsupported"
            starts.append(i)
            bases.append(base)
            i = j
        starts.append(n)
        nchunks = len(bases)

        col_of_slot = np.full(U_PER, -1, np.int64)
        for c in range(nchunks):
            s0, s1 = starts[c], starts[c + 1]
            slots = np.unique(su_l[s0:s1])
            assert slots.max() - bases[c] < WSLOTS
            col_of_slot[slots] = WSLOTS * c + (slots - bases[c])
        uncovered = np.flatnonzero(col_of_slot < 0)
        cores.append(dict(lo=lo, hi=hi, su_l=su_l, starts=starts, bases=bases,
                          nchunks=nchunks, col_of_slot=col_of_slot,
                          uncovered=uncovered))

    need = max(c["nchunks"] + (len(c["uncovered"]) + WSLOTS - 1) // WSLOTS + 1
               for c in cores)
    B = need + (-need) % SB          # superblocks of SB chunks
    NE = B * CHUNK
    C = B * WSLOTS

    per_core = []
    for k in range(NCORES):
        ci = cores[k]
        lo, hi = ci["lo"], ci["hi"]
        su_l, starts, bases = ci["su_l"], ci["starts"], ci["bases"]
        nchunks = ci["nchunks"]

        col_of_slot = ci["col_of_slot"].copy()
        unc = ci["uncovered"]
        if len(unc):
            cols = WSLOTS * nchunks + np.arange(len(unc))
            assert cols.max() < C
            col_of_slot[unc] = cols
        assert (col_of_slot >= 0).all()

        x_T = np.zeros((128, NE), np.float32)      # [uT ; vT]
        # e4: 4 chunks side by side in rows 0:16 / 32:48 / 64:80 / 96:112
        e4_T = np.zeros((112, NE // SB), np.float32)
        slot_cols = np.full((GP, GROUPS * B), -1.0, np.float32)
        n = hi - lo
        if n:
            x_src = np.empty((128, n), np.float32)
            x_src[0:64] = u[su[lo:hi]].T
            x_src[64:128] = v[sv[lo:hi]].T
            e_src = se[lo:hi].T                     # [16, n]
        for c in range(nchunks):
            s0, s1 = starts[c], starts[c + 1]
            m = s1 - s0
            x_T[:, c * CHUNK:c * CHUNK + m] = x_src[:, s0:s1]
            r0 = 32 * (c % SB)
            col0 = (c // SB) * CHUNK
            e4_T[r0:r0 + 16, col0:col0 + m] = e_src[:, s0:s1]
            full = np.full(CHUNK, -1.0, np.float32)
            full[:m] = (su_l[s0:s1] - bases[c]).astype(np.float32)
            slot_cols[:, GROUPS * c:GROUPS * (c + 1)] = \
                full.reshape(GROUPS, GP).T

        u_T_compact = np.zeros((64, C), np.float32)
        u_T_compact[:, col_of_slot] = u[k * U_PER:(k + 1) * U_PER].T

        per_core.append(dict(x_T=x_T, e4_T=e4_T, slot_cols=slot_cols,
                             u_T_compact=u_T_compact,
                             col_of_slot=col_of_slot))
    return per_core, B, NE, C


# ---------------------------------------------------------------- device side

def _build_program(B, NE, C, io_dtype_np):
    import concourse.bacc as bacc
    import concourse.mybir as mybir
    import concourse.tile as tile

    FB = (C + 511) // 512               # f-MLP chunks
    md = mybir.dt.from_np(np.dtype(io_dtype_np))
    f32 = mybir.dt.float32
    Relu = mybir.ActivationFunctionType.Relu
    Copy = mybir.ActivationFunctionType.Copy
    Alu = mybir.AluOpType

    nc = bacc.Bacc("TRN2", target_bir_lowering=False, debug=False,
                   num_devices=NCORES)

    # I/O
    x_T = nc.dram_tensor("x_T", [128, NE], md, kind="ExternalInput")
    e4_T = nc.dram_tensor("e4_T", [112, NE // SB], md, kind="ExternalInput")
    slot_cols = nc.dram_tensor("slot_cols", [GP, GROUPS * B], md,
                               kind="ExternalInput")
    u_Tc = nc.dram_tensor("u_Tc", [64, C], md, kind="ExternalInput")
    w1a = nc.dram_tensor("w1a", [128, 128], md, kind="ExternalInput")
    w1e4 = nc.dram_tensor("w1e4", [112, 128], md, kind="ExternalInput")
    w2 = nc.dram_tensor("w2", [128, 64], md, kind="ExternalInput")
    fw1 = nc.dram_tensor("fw1", [128, 128], md, kind="ExternalInput")
    fw2 = nc.dram_tensor("fw2", [128, 64], md, kind="ExternalInput")
    b1 = nc.dram_tensor("b1", [128, 1], f32, kind="ExternalInput")
    b2m = nc.dram_tensor("b2m", [GP, 64 * GROUPS], f32, kind="ExternalInput")
    fb1 = nc.dram_tensor("fb1", [128, 1], f32, kind="ExternalInput")
    fb2 = nc.dram_tensor("fb2", [64, 1], f32, kind="ExternalInput")
    iota16 = nc.dram_tensor("iota16", [GP, 64 * GROUPS * SB], md,
                            kind="ExternalInput")
    out_T = nc.dram_tensor("out_T", [64, C], f32, kind="ExternalOutput")

    NSB = B // SB                       # superblocks

    with tile.TileContext(nc) as tc:
        with (
            tc.tile_pool(name="consts", bufs=1) as cp,
            tc.tile_pool(name="xf", bufs=1) as xfp,
            tc.tile_pool(name="xin", bufs=3) as xp,
            tc.tile_pool(name="work", bufs=4) as wp,
            tc.tile_pool(name="ps1", bufs=4, space="PSUM") as p1,
            tc.tile_pool(name="ps2", bufs=2, space="PSUM") as p2,
            tc.tile_pool(name="ps3", bufs=1, space="PSUM") as p3,
            tc.tile_pool(name="psf", bufs=1, space="PSUM") as pf,
        ):
            # resident constants
            w1a_s = cp.tile([128, 128], md)
            w1e4_s = cp.tile([112, 128], md)
            w2_s = cp.tile([128, 64], md)
            fw1_s = cp.tile([128, 128], md)
            fw2_s = cp.tile([128, 64], md)
            b1_s = cp.tile([128, 1], f32)
            b2m_s = cp.tile([GP, 64 * GROUPS], f32)
            fb1_s = cp.tile([128, 1], f32)
            fb2_s = cp.tile([64, 1], f32)
            iota_s = cp.tile([GP, 64 * GROUPS * SB], md)
            slot_s = cp.tile([GP, GROUPS * B], md)
            for dst, src in [(w1a_s, w1a), (w1e4_s, w1e4), (w2_s, w2),
                             (fw1_s, fw1), (fw2_s, fw2), (b1_s, b1),
                             (b2m_s, b2m), (fb1_s, fb1), (fb2_s, fb2),
                             (iota_s, iota16), (slot_s, slot_cols)]:
                nc.sync.dma_start(dst[:], src[:])

            # xf: rows 0:64 aggT (flushed per superblock), rows 64:128 uT
            xf = xfp.tile([128, C], md)
            nc.sync.dma_start(xf[64:128, :], u_Tc[:])

            # f-MLP chunk emitter (interleaved into the main loop)
            f_done = [0]

            def emit_f(fc):
                w = min(512, C - 512 * fc)
                fsl = slice(512 * fc, 512 * fc + w)
                zf1 = pf.tile([128, 512], f32, tag="zf")
                nc.tensor.matmul(zf1[:, :w], lhsT=fw1_s[:], rhs=xf[:, fsl],
                                 start=True, stop=True)
                hf = wp.tile([128, 512], md, tag="hf")
                nc.scalar.activation(hf[:, :w], zf1[:, :w], Relu,
                                     bias=fb1_s[:])
                zf2 = pf.tile([64, 512], f32, tag="zf")
                nc.tensor.matmul(zf2[:, :w], lhsT=fw2_s[:], rhs=hf[:, :w],
                                 start=True, stop=True)
                ot = wp.tile([64, 512], f32, tag="ot")
                nc.scalar.activation(ot[:, :w], zf2[:, :w], Relu,
                                     bias=fb2_s[:])
                nc.sync.dma_start(out_T[:, fsl], ot[:, :w])
                f_done[0] = fc + 1

            def make_oh(m):
                oh = wp.tile([GP, 64 * GROUPS * SB], md, tag="oh", name="oh")
                nc.vector.tensor_tensor(
                    oh[:].rearrange("p (g s) -> p g s", g=GROUPS * SB),
                    iota_s[:].rearrange("p (g s) -> p g s", g=GROUPS * SB),
                    slot_s[:, GROUPS * SB * m:GROUPS * SB * (m + 1)]
                        [:, :, None]
                        .to_broadcast([GP, GROUPS * SB, WSLOTS]),
                    op=Alu.is_equal)
                return oh

            oh_next = make_oh(0)

            for m in range(NSB):                    # superblock = 4 chunks
                if m % 2 == 0:                      # 2-superblock x1 batch
                    nb = min(2, NSB - m)
                    x1 = xp.tile([128, 2 * SB * CHUNK], md, tag="x1")
                    nc.sync.dma_start(
                        x1[:, :nb * SB * CHUNK],
                        x_T[:, m * SB * CHUNK:(m + nb) * SB * CHUNK])
                x1off = (m % 2) * SB * CHUNK
                e4 = xp.tile([112, CHUNK], md, tag="e4")
                nc.sync.dma_start(e4[:], e4_T[:, m * CHUNK:(m + 1) * CHUNK])

                oh = oh_next

                # L1 per chunk, column-split into feature halves so the thin
                # K=16 e matmul streams concurrently with the w1a half that
                # targets the other PE column group:
                #   (e_lo || w1a_hi) then (w1a_lo || e_hi)
                z1s = [p1.tile([128, CHUNK], f32, tag="z1", name="z1")
                       for _ in range(SB)]
                for q in range(SB):
                    z1 = z1s[q]
                    r0 = 32 * q
                    xsl = x1[:, x1off + q * CHUNK:x1off + (q + 1) * CHUNK]
                    nc.tensor.matmul(z1[0:64, :],
                                     lhsT=w1e4_s[r0:r0 + 16, 0:64],
                                     rhs=e4[r0:r0 + 16, :],
                                     start=True, stop=False,
                                     tile_position=(r0, 0),
                                     skip_group_check=True)
                    nc.tensor.matmul(z1[64:128, :], lhsT=w1a_s[:, 64:128],
                                     rhs=xsl,
                                     start=True, stop=False,
                                     tile_position=(0, 64),
                                     skip_group_check=True)
                    nc.tensor.matmul(z1[0:64, :], lhsT=w1a_s[:, 0:64],
                                     rhs=xsl,
                                     start=False, stop=True,
                                     tile_position=(0, 0),
                                     skip_group_check=True)
                    nc.tensor.matmul(z1[64:128, :],
                                     lhsT=w1e4_s[r0:r0 + 16, 64:128],
                                     rhs=e4[r0:r0 + 16, :],
                                     start=False, stop=True,
                                     tile_position=(r0, 64),
                                     skip_group_check=True)

                # dual scatter accumulators: rows 0:64 take groups 0,1 and
                # rows 64:128 take groups 2,3 (distinct PE column groups ->
                # the per-chunk scatter matmuls run in concurrent pairs)
                pT = p3.tile([128, SB * WSLOTS], f32, tag="pT")
                for q in range(SB):                 # per-chunk tail
                    h1 = wp.tile([128, CHUNK], md, tag="h1")
                    nc.scalar.activation(h1[:], z1s[q][:], Relu, bias=b1_s[:])

                    # L2 in column-split halves: edges 0:64 of each group to
                    # PE col groups {0,1}, edges 64:128 to {2,3} (concurrent)
                    z2 = p2.tile([GP, 64 * GROUPS], f32, tag="z2")
                    for g in range(GROUPS):
                        nc.tensor.matmul(z2[0:64, 64 * g:64 * (g + 1)],
                                         lhsT=h1[:, GP * g:GP * g + 64],
                                         rhs=w2_s[:], start=True, stop=True,
                                         tile_position=(0, 0),
                                         skip_group_check=True)
                        nc.tensor.matmul(z2[64:128, 64 * g:64 * (g + 1)],
                                         lhsT=h1[:, GP * g + 64:
                                                GP * (g + 1)],
                                         rhs=w2_s[:], start=True, stop=True,
                                         tile_position=(0, 64),
                                         skip_group_check=True)
                    h2T = wp.tile([GP, 64 * GROUPS], md, tag="h2T")
                    nc.vector.tensor_tensor(h2T[:], z2[:], b2m_s[:],
                                            op=Alu.add)
                    nc.vector.tensor_scalar_max(h2T[:], h2T[:], 0.0)

                    for gh in range(2):             # pairs (0,2) and (1,3)
                        g0, g1 = gh, gh + 2
                        nc.tensor.matmul(pT[0:64, 64 * q:64 * (q + 1)],
                                         lhsT=h2T[:, 64 * g0:64 * (g0 + 1)],
                                         rhs=oh[:, 256 * q + 64 * g0:
                                                256 * q + 64 * (g0 + 1)],
                                         start=(gh == 0), stop=(gh == 1),
                                         tile_position=(0, 0),
                                         skip_group_check=True)
                        nc.tensor.matmul(pT[64:128, 64 * q:64 * (q + 1)],
                                         lhsT=h2T[:, 64 * g1:64 * (g1 + 1)],
                                         rhs=oh[:, 256 * q + 64 * g1:
                                                256 * q + 64 * (g1 + 1)],
                                         start=(gh == 0), stop=(gh == 1),
                                         tile_position=(0, 64),
                                         skip_group_check=True)
                # flush: xf = pT_low + pT_high (DVE reads at most one PSUM
                # operand per op, so stage the low half through ACT first)
                ptmp = wp.tile([64, SB * WSLOTS], f32, tag="ptmp")
                nc.scalar.activation(ptmp[:], pT[0:64, :], Copy)
                nc.vector.tensor_tensor(
                    xf[0:64, SB * WSLOTS * m:SB * WSLOTS * (m + 1)],
                    ptmp[:], pT[64:128, :], op=Alu.add)
                if m + 1 < NSB:                     # prefetch next one-hot
                    oh_next = make_oh(m + 1)
                while (f_done[0] + 1) * 512 <= (m + 1) * SB * WSLOTS:
                    emit_f(f_done[0])

            for fc in range(f_done[0], FB):
                emit_f(fc)

    nc.compile()
    return nc


def _make_in_maps(per_core, consts, io_dtype_np):
    in_maps = []
    for pc in per_core:
        m = dict(
            x_T=pc["x_T"].astype(io_dtype_np),
            e4_T=pc["e4_T"].astype(io_dtype_np),
            slot_cols=pc["slot_cols"].astype(io_dtype_np),
            u_Tc=pc["u_T_compact"].astype(io_dtype_np),
            **{k: v for k, v in consts.items()},
        )
        in_maps.append(m)
    return in_maps


def _make_consts(g_w1, g_b1, g_w2, g_b2, f_w1, f_b1, f_w2, f_b2, io_dtype_np):
    dt = io_dtype_np
    g_w1 = np.asarray(g_w1, np.float32)
    w1e4 = np.zeros((112, 128), np.float32)
    for q in range(SB):
        w1e4[32 * q:32 * q + 16] = g_w1[128:144]
    # f-MLP input is [aggT ; uT] (agg rows first), so permute f_w1 rows
    f_w1 = np.asarray(f_w1, np.float32)
    f_w1p = np.concatenate([f_w1[64:128], f_w1[0:64]], axis=0)
    return dict(
        w1a=np.ascontiguousarray(g_w1[0:128]).astype(dt),
        w1e4=w1e4.astype(dt),
        w2=np.asarray(g_w2, np.float32).astype(dt),
        fw1=np.ascontiguousarray(f_w1p).astype(dt),
        fw2=np.asarray(f_w2, np.float32).astype(dt),
        b1=np.asarray(g_b1, np.float32).reshape(128, 1),
        b2m=np.ascontiguousarray(
            np.tile(np.asarray(g_b2, np.float32)[None, :], (GP, GROUPS))),
        fb1=np.asarray(f_b1, np.float32).reshape(128, 1),
        fb2=np.asarray(f_b2, np.float32).reshape(64, 1),
        iota16=np.ascontiguousarray(
            np.tile(np.arange(WSLOTS, dtype=np.float32)[None, :],
                    (GP, GROUPS * SB))).astype(dt),
    )


_last_run_info = {}


def kernel(u, v, e_vals, e_idx_v, e_idx_u, g_w1, g_b1, g_w2, g_b2,
           f_w1, f_b1, f_w2, f_b2, _trace=False):
    import ml_dtypes
    from concourse import bass_utils

    io_dtype_np = ml_dtypes.bfloat16

    per_core, B, NE, C = _preprocess(u, v, e_vals, e_idx_v, e_idx_u)
    consts = _make_consts(g_w1, g_b1, g_w2, g_b2, f_w1, f_b1, f_w2, f_b2,
                          io_dtype_np)
    nc = _build_program(B, NE, C, io_dtype_np)
    in_maps = _make_in_maps(per_core, consts, io_dtype_np)

    res = bass_utils.run_bass_kernel_spmd(
        nc, in_maps, core_ids=list(range(NCORES)), trace=_trace)
    _last_run_info.clear()
    _last_run_info.update(B=B, NE=NE, C=C, res=res)

    out = np.zeros((U, 64), np.float32)
    for k in range(NCORES):
        out_T = res.results[k]["out_T"]
        cols = per_core[k]["col_of_slot"]
        out[k * U_PER:(k + 1) * U_PER] = out_T[:, cols].T
    return out


# revision 16
# speedup vs baseline: 1.0909x; 1.0909x over previous
"""Trainium2 Bass kernel for nn_HalfConv_876173328516 (GNN message passing).

Strategy
--------
Host: sort edges by e_idx_u; core k owns u rows [k*6250, (k+1)*6250), so the 8
cores are fully independent (no collectives). Per-edge inputs are expanded and
transposed on the host into a [128, NE] u||v feature stream plus a packed
[112, NE/4] e_vals stream per core (4 chunks side by side in PE row groups
0-15/32-47/64-79/96-111), with edges packed into 512-edge chunks that
(a) never split one u across chunks and (b) span < 64 u-slots.

Device (per core, per superblock of 4 chunks = 2048 edges):
  L1   z1[q] = W1uv.T @ x_q       (4 matmuls, one w1a weight-load context)
       z1[q] += W1e.T @ e_q       (4 K=16 matmuls packed in disjoint PE row
                                   groups -> run concurrently)
       h1 = relu(z1 + b1)         (ACT, -> SBUF bf16)
  L2   per 128-edge group: z2 = h1_g.T @ W2    (edge-major output)
       h2 = relu(z2 + b2)                      (DVE add + max)
  SUM  one-hot[e, slot] built once per superblock ([128, 1024] is_equal)
       pT[64 feats, 64 slots/chunk] += h2_g.T @ oh_g  (PSUM accumulate)
       flush pT -> xf[0:64, 256*m4:...]        (one DVE copy per superblock)
  f-MLP over all compact slot columns: xf = [aggT ; uT], two matmuls + relus.

Host: out[u] = out_T[:, col_of_slot[u]].T per core.
"""

import numpy as np

U, V, E = 50000, 50000, 800000
NCORES = 8
U_PER = U // NCORES          # 6250
CHUNK = 512                  # edges per chunk
GP = 128                     # edges per matmul group
GROUPS = CHUNK // GP         # 4
WSLOTS = 64                  # slot window per chunk
SB = 4                       # chunks per superblock


# ---------------------------------------------------------------- host side

def _preprocess(u, v, e_vals, e_idx_v, e_idx_u):
    u = np.ascontiguousarray(np.asarray(u, np.float32))
    v = np.ascontiguousarray(np.asarray(v, np.float32))
    e_vals = np.ascontiguousarray(np.asarray(e_vals, np.float32))
    e_idx_u = np.asarray(e_idx_u).astype(np.int64)
    e_idx_v = np.asarray(e_idx_v).astype(np.int64)

    perm = np.argsort(e_idx_u, kind="stable")
    su = e_idx_u[perm]
    sv = e_idx_v[perm]
    se = e_vals[perm]

    core_lo = np.searchsorted(su, np.arange(NCORES) * U_PER, side="left")
    core_hi = np.searchsorted(su, (np.arange(NCORES) + 1) * U_PER, side="left")

    cores = []
    for k in range(NCORES):
        lo, hi = int(core_lo[k]), int(core_hi[k])
        su_l = (su[lo:hi] - k * U_PER).astype(np.int64)
        n = hi - lo
        starts, bases = [], []
        i = 0
        while i < n:
            base = int(su_l[i])
            j = min(i + CHUNK, n)
            j = min(j, int(np.searchsorted(su_l, base + WSLOTS, side="left")))
            if j < n:
                # step back to a u-boundary so no u straddles chunks
                j2 = int(np.searchsorted(su_l, su_l[j - 1], side="left"))
                if j2 > i and su_l[j - 1] == su_l[j]:
                    j = j2
            assert j > i, "u degree >= CHUNK unsupported"
            starts.append(i)
            bases.append(base)
            i = j
        starts.append(n)
        nchunks = len(bases)

        col_of_slot = np.full(U_PER, -1, np.int64)
        for c in range(nchunks):
            s0, s1 = starts[c], starts[c + 1]
            slots = np.unique(su_l[s0:s1])
            assert slots.max() - bases[c] < WSLOTS
            col_of_slot[slots] = WSLOTS * c + (slots - bases[c])
        uncovered = np.flatnonzero(col_of_slot < 0)
        cores.append(dict(lo=lo, hi=hi, su_l=su_l, starts=starts, bases=bases,
                          nchunks=nchunks, col_of_slot=col_of_slot,
                          uncovered=uncovered))

    need = max(c["nchunks"] + (len(c["uncovered"]) + WSLOTS - 1) // WSLOTS + 1
               for c in cores)
    B = need + (-need) % SB          # superblocks of SB chunks
    NE = B * CHUNK
    C = B * WSLOTS

    per_core = []
    for k in range(NCORES):
        ci = cores[k]
        lo, hi = ci["lo"], ci["hi"]
        su_l, starts, bases = ci["su_l"], ci["starts"], ci["bases"]
        nchunks = ci["nchunks"]

        col_of_slot = ci["col_of_slot"].copy()
        unc = ci["uncovered"]
        if len(unc):
            cols = WSLOTS * nchunks + np.arange(len(unc))
            assert cols.max() < C
            col_of_slot[unc] = cols
        assert (col_of_slot >= 0).all()

        x_T = np.zeros((128, NE), np.float32)      # [uT ; vT]
        # e4: 4 chunks side by side in rows 0:16 / 32:48 / 64:80 / 96:112
        e4_T = np.zeros((112, NE // SB), np.float32)
        slot_cols = np.full((GP, GROUPS * B), -1.0, np.float32)
        n = hi - lo
        if n:
            x_src = np.empty((128, n), np.float32)
            x_src[0:64] = u[su[lo:hi]].T
            x_src[64:128] = v[sv[lo:hi]].T
            e_src = se[lo:hi].T                     # [16, n]
        for c in range(nchunks):
            s0, s1 = starts[c], starts[c + 1]
            m = s1 - s0
            x_T[:, c * CHUNK:c * CHUNK + m] = x_src[:, s0:s1]
            r0 = 32 * (c % SB)
            col0 = (c // SB) * CHUNK
            e4_T[r0:r0 + 16, col0:col0 + m] = e_src[:, s0:s1]
            full = np.full(CHUNK, -1.0, np.float32)
            full[:m] = (su_l[s0:s1] - bases[c]).astype(np.float32)
            slot_cols[:, GROUPS * c:GROUPS * (c + 1)] = \
                full.reshape(GROUPS, GP).T

        u_T_compact = np.zeros((64, C), np.float32)
        u_T_compact[:, col_of_slot] = u[k * U_PER:(k + 1) * U_PER].T

        per_core.append(dict(x_T=x_T, e4_T=e4_T, slot_cols=slot_cols,
                             u_T_compact=u_T_compact,
                             col_of_slot=col_of_slot))
    return per_core, B, NE, C


# ---------------------------------------------------------------- device side

def _build_program(B, NE, C, io_dtype_np):
    import concourse.bacc as bacc
    import concourse.mybir as mybir
    import concourse.tile as tile

    FB = (C + 511) // 512               # f-MLP chunks
    md = mybir.dt.from_np(np.dtype(io_dtype_np))
    f32 = mybir.dt.float32
    Relu = mybir.ActivationFunctionType.Relu
    Copy = mybir.ActivationFunctionType.Copy
    Alu = mybir.AluOpType

    nc = bacc.Bacc("TRN2", target_bir_lowering=False, debug=False,
                   num_devices=NCORES)

    # I/O
    x_T = nc.dram_tensor("x_T", [128, NE], md, kind="ExternalInput")
    e4_T = nc.dram_tensor("e4_T", [112, NE // SB], md, kind="ExternalInput")
    slot_cols = nc.dram_tensor("slot_cols", [GP, GROUPS * B], md,
                               kind="ExternalInput")
    u_Tc = nc.dram_tensor("u_Tc", [64, C], md, kind="ExternalInput")
    w1a = nc.dram_tensor("w1a", [128, 128], md, kind="ExternalInput")
    w1e4 = nc.dram_tensor("w1e4", [112, 128], md, kind="ExternalInput")
    w2 = nc.dram_tensor("w2", [128, 64], md, kind="ExternalInput")
    fw1 = nc.dram_tensor("fw1", [128, 128], md, kind="ExternalInput")
    fw2 = nc.dram_tensor("fw2", [128, 64], md, kind="ExternalInput")
    b1 = nc.dram_tensor("b1", [128, 1], f32, kind="ExternalInput")
    b2m = nc.dram_tensor("b2m", [GP, 64 * GROUPS], f32, kind="ExternalInput")
    fb1 = nc.dram_tensor("fb1", [128, 1], f32, kind="ExternalInput")
    fb2 = nc.dram_tensor("fb2", [64, 1], f32, kind="ExternalInput")
    iota16 = nc.dram_tensor("iota16", [GP, 64 * GROUPS * SB], md,
                            kind="ExternalInput")
    out_T = nc.dram_tensor("out_T", [64, C], f32, kind="ExternalOutput")

    NSB = B // SB                       # superblocks

    with tile.TileContext(nc) as tc:
        with (
            tc.tile_pool(name="consts", bufs=1) as cp,
            tc.tile_pool(name="xf", bufs=1) as xfp,
            tc.tile_pool(name="xin", bufs=3) as xp,
            tc.tile_pool(name="work", bufs=4) as wp,
            tc.tile_pool(name="ps1", bufs=4, space="PSUM") as p1,
            tc.tile_pool(name="ps2", bufs=2, space="PSUM") as p2,
            tc.tile_pool(name="ps3", bufs=1, space="PSUM") as p3,
            tc.tile_pool(name="psf", bufs=1, space="PSUM") as pf,
        ):
            # resident constants
            w1a_s = cp.tile([128, 128], md)
            w1e4_s = cp.tile([112, 128], md)
            w2_s = cp.tile([128, 64], md)
            fw1_s = cp.tile([128, 128], md)
            fw2_s = cp.tile([128, 64], md)
            b1_s = cp.tile([128, 1], f32)
            b2m_s = cp.tile([GP, 64 * GROUPS], f32)
            fb1_s = cp.tile([128, 1], f32)
            fb2_s = cp.tile([64, 1], f32)
            iota_s = cp.tile([GP, 64 * GROUPS * SB], md)
            slot_s = cp.tile([GP, GROUPS * B], md)
            for dst, src in [(w1a_s, w1a), (w1e4_s, w1e4), (w2_s, w2),
                             (fw1_s, fw1), (fw2_s, fw2), (b1_s, b1),
                             (b2m_s, b2m), (fb1_s, fb1), (fb2_s, fb2),
                             (iota_s, iota16), (slot_s, slot_cols)]:
                nc.sync.dma_start(dst[:], src[:])

            # xf: rows 0:64 aggT (flushed per superblock), rows 64:128 uT
            xf = xfp.tile([128, C], md)
            nc.sync.dma_start(xf[64:128, :], u_Tc[:])

            # f-MLP chunk emitter (interleaved into the main loop)
            f_done = [0]

            def emit_f(fc):
                w = min(512, C - 512 * fc)
                fsl = slice(512 * fc, 512 * fc + w)
                zf1 = pf.tile([128, 512], f32, tag="zf")
                nc.tensor.matmul(zf1[:, :w], lhsT=fw1_s[:], rhs=xf[:, fsl],
                                 start=True, stop=True)
                hf = wp.tile([128, 512], md, tag="hf")
                nc.scalar.activation(hf[:, :w], zf1[:, :w], Relu,
                                     bias=fb1_s[:])
                zf2 = pf.tile([64, 512], f32, tag="zf")
                nc.tensor.matmul(zf2[:, :w], lhsT=fw2_s[:], rhs=hf[:, :w],
                                 start=True, stop=True)
                ot = wp.tile([64, 512], f32, tag="ot")
                nc.scalar.activation(ot[:, :w], zf2[:, :w], Relu,
                                     bias=fb2_s[:])
                nc.sync.dma_start(out_T[:, fsl], ot[:, :w])
                f_done[0] = fc + 1

            def make_oh(m):
                oh = wp.tile([GP, 64 * GROUPS * SB], md, tag="oh", name="oh")
                nc.vector.tensor_tensor(
                    oh[:].rearrange("p (g s) -> p g s", g=GROUPS * SB),
                    iota_s[:].rearrange("p (g s) -> p g s", g=GROUPS * SB),
                    slot_s[:, GROUPS * SB * m:GROUPS * SB * (m + 1)]
                        [:, :, None]
                        .to_broadcast([GP, GROUPS * SB, WSLOTS]),
                    op=Alu.is_equal)
                return oh

            oh_next = make_oh(0)

            for m in range(NSB):                    # superblock = 4 chunks
                if m % 2 == 0:                      # 2-superblock x1 batch
                    nb = min(2, NSB - m)
                    x1 = xp.tile([128, 2 * SB * CHUNK], md, tag="x1")
                    nc.sync.dma_start(
                        x1[:, :nb * SB * CHUNK],
                        x_T[:, m * SB * CHUNK:(m + nb) * SB * CHUNK])
                x1off = (m % 2) * SB * CHUNK
                e4 = xp.tile([112, CHUNK], md, tag="e4")
                nc.sync.dma_start(e4[:], e4_T[:, m * CHUNK:(m + 1) * CHUNK])

                oh = oh_next

                # L1 per chunk: K=16 e matmul in a PE row group, then w1a
                z1s = [p1.tile([128, CHUNK], f32, tag="z1", name="z1")
                       for _ in range(SB)]
                for q in range(SB):
                    z1 = z1s[q]
                    r0 = 32 * q
                    xsl = x1[:, x1off + q * CHUNK:x1off + (q + 1) * CHUNK]
                    nc.tensor.matmul(z1[:], lhsT=w1e4_s[r0:r0 + 16, :],
                                     rhs=e4[r0:r0 + 16, :],
                                     start=True, stop=False,
                                     tile_position=(r0, 0),
                                     skip_group_check=True)
                    nc.tensor.matmul(z1[:], lhsT=w1a_s[:],
                                     rhs=xsl,
                                     start=False, stop=True,
                                     skip_group_check=True)

                # dual scatter accumulators: rows 0:64 take groups 0,1 and
                # rows 64:128 take groups 2,3 (distinct PE column groups ->
                # the per-chunk scatter matmuls run in concurrent pairs)
                pT = p3.tile([128, SB * WSLOTS], f32, tag="pT")
                for q in range(SB):                 # per-chunk tail
                    h1 = wp.tile([128, CHUNK], md, tag="h1")
                    nc.scalar.activation(h1[:], z1s[q][:], Relu, bias=b1_s[:])

                    # L2 in column-split halves: edges 0:64 of each group to
                    # PE col groups {0,1}, edges 64:128 to {2,3} (concurrent)
                    z2 = p2.tile([GP, 64 * GROUPS], f32, tag="z2")
                    for g in range(GROUPS):
                        nc.tensor.matmul(z2[0:64, 64 * g:64 * (g + 1)],
                                         lhsT=h1[:, GP * g:GP * g + 64],
                                         rhs=w2_s[:], start=True, stop=True,
                                         tile_position=(0, 0),
                                         skip_group_check=True)
                        nc.tensor.matmul(z2[64:128, 64 * g:64 * (g + 1)],
                                         lhsT=h1[:, GP * g + 64:
                                                GP * (g + 1)],
                                         rhs=w2_s[:], start=True, stop=True,
                                         tile_position=(0, 64),
                                         skip_group_check=True)
                    h2T = wp.tile([GP, 64 * GROUPS], md, tag="h2T")
                    nc.vector.tensor_tensor(h2T[:], z2[:], b2m_s[:],
                                            op=Alu.add)
                    nc.vector.tensor_scalar_max(h2T[:], h2T[:], 0.0)

                    for gh in range(2):             # pairs (0,2) and (1,3)
                        g0, g1 = gh, gh + 2
                        nc.tensor.matmul(pT[0:64, 64 * q:64 * (q + 1)],
                                         lhsT=h2T[:, 64 * g0:64 * (g0 + 1)],
                                         rhs=oh[:, 256 * q + 64 * g0:
                                                256 * q + 64 * (g0 + 1)],
                                         start=(gh == 0), stop=(gh == 1),
                                         tile_position=(0, 0),
                                         skip_group_check=True)
                        nc.tensor.matmul(pT[64:128, 64 * q:64 * (q + 1)],
                                         lhsT=h2T[:, 64 * g1:64 * (g1 + 1)],
                                         rhs=oh[:, 256 * q + 64 * g1:
                                                256 * q + 64 * (g1 + 1)],
                                         start=(gh == 0), stop=(gh == 1),
                                         tile_position=(0, 64),
                                         skip_group_check=True)
                # flush: xf = pT_low + pT_high (DVE reads at most one PSUM
                # operand per op, so stage the low half through ACT first)
                ptmp = wp.tile([64, SB * WSLOTS], f32, tag="ptmp")
                nc.scalar.activation(ptmp[:], pT[0:64, :], Copy)
                nc.vector.tensor_tensor(
                    xf[0:64, SB * WSLOTS * m:SB * WSLOTS * (m + 1)],
                    ptmp[:], pT[64:128, :], op=Alu.add)
                if m + 1 < NSB:                     # prefetch next one-hot
                    oh_next = make_oh(m + 1)
                while (f_done[0] + 1) * 512 <= (m + 1) * SB * WSLOTS:
                    emit_f(f_done[0])

            for fc in range(f_done[0], FB):
                emit_f(fc)

    nc.compile()
    return nc


def _make_in_maps(per_core, consts, io_dtype_np):
    in_maps = []
    for pc in per_core:
        m = dict(
            x_T=pc["x_T"].astype(io_dtype_np),
            e4_T=pc["e4_T"].astype(io_dtype_np),
            slot_cols=pc["slot_cols"].astype(io_dtype_np),
            u_Tc=pc["u_T_compact"].astype(io_dtype_np),
            **{k: v for k, v in consts.items()},
        )
        in_maps.append(m)
    return in_maps


def _make_consts(g_w1, g_b1, g_w2, g_b2, f_w1, f_b1, f_w2, f_b2, io_dtype_np):
    dt = io_dtype_np
    g_w1 = np.asarray(g_w1, np.float32)
    w1e4 = np.zeros((112, 128), np.float32)
    for q in range(SB):
        w1e4[32 * q:32 * q + 16] = g_w1[128:144]
    # f-MLP input is [aggT ; uT] (agg rows first), so permute f_w1 rows
    f_w1 = np.asarray(f_w1, np.float32)
    f_w1p = np.concatenate([f_w1[64:128], f_w1[0:64]], axis=0)
    return dict(
        w1a=np.ascontiguousarray(g_w1[0:128]).astype(dt),
        w1e4=w1e4.astype(dt),
        w2=np.asarray(g_w2, np.float32).astype(dt),
        fw1=np.ascontiguousarray(f_w1p).astype(dt),
        fw2=np.asarray(f_w2, np.float32).astype(dt),
        b1=np.asarray(g_b1, np.float32).reshape(128, 1),
        b2m=np.ascontiguousarray(
            np.tile(np.asarray(g_b2, np.float32)[None, :], (GP, GROUPS))),
        fb1=np.asarray(f_b1, np.float32).reshape(128, 1),
        fb2=np.asarray(f_b2, np.float32).reshape(64, 1),
        iota16=np.ascontiguousarray(
            np.tile(np.arange(WSLOTS, dtype=np.float32)[None, :],
                    (GP, GROUPS * SB))).astype(dt),
    )


_last_run_info = {}


def kernel(u, v, e_vals, e_idx_v, e_idx_u, g_w1, g_b1, g_w2, g_b2,
           f_w1, f_b1, f_w2, f_b2, _trace=False):
    import ml_dtypes
    from concourse import bass_utils

    io_dtype_np = ml_dtypes.bfloat16

    per_core, B, NE, C = _preprocess(u, v, e_vals, e_idx_v, e_idx_u)
    consts = _make_consts(g_w1, g_b1, g_w2, g_b2, f_w1, f_b1, f_w2, f_b2,
                          io_dtype_np)
    nc = _build_program(B, NE, C, io_dtype_np)
    in_maps = _make_in_maps(per_core, consts, io_dtype_np)

    res = bass_utils.run_bass_kernel_spmd(
        nc, in_maps, core_ids=list(range(NCORES)), trace=_trace)
    _last_run_info.clear()
    _last_run_info.update(B=B, NE=NE, C=C, res=res)

    out = np.zeros((U, 64), np.float32)
    for k in range(NCORES):
        out_T = res.results[k]["out_T"]
        cols = per_core[k]["col_of_slot"]
        out[k * U_PER:(k + 1) * U_PER] = out_T[:, cols].T
    return out


# revision 18
# speedup vs baseline: 1.4282x; 1.3092x over previous
"""Trainium2 Bass kernel for nn_HalfConv_876173328516 (GNN message passing).

Strategy
--------
Host: sort edges by e_idx_u; core k owns u rows [k*6250, (k+1)*6250), so the 8
cores are fully independent (no collectives). Per-edge inputs are expanded and
transposed on the host into a [128, NE] u||v feature stream plus a packed
[112, NE/4] e_vals stream per core (4 chunks side by side in PE row groups
0-15/32-47/64-79/96-111), with edges packed into 512-edge chunks that
(a) never split one u across chunks and (b) span < 64 u-slots.

Device (per core, per superblock of 4 chunks = 2048 edges):
  L1   z1[q] = W1uv.T @ x_q       (4 matmuls, one w1a weight-load context)
       z1[q] += W1e.T @ e_q       (4 K=16 matmuls packed in disjoint PE row
                                   groups -> run concurrently)
       h1 = relu(z1 + b1)         (ACT, -> SBUF bf16)
  L2   per 128-edge group: z2 = h1_g.T @ W2    (edge-major output)
       h2 = relu(z2 + b2)                      (DVE add + max)
  SUM  one-hot[e, slot] built once per superblock ([128, 1024] is_equal)
       pT[64 feats, 64 slots/chunk] += h2_g.T @ oh_g  (PSUM accumulate)
       flush pT -> xf[0:64, 256*m4:...]        (one DVE copy per superblock)
  f-MLP over all compact slot columns: xf = [aggT ; uT], two matmuls + relus.

Host: out[u] = out_T[:, col_of_slot[u]].T per core.
"""

import numpy as np

U, V, E = 50000, 50000, 800000
NCORES = 8
U_PER = U // NCORES          # 6250
CHUNK = 512                  # edges per chunk
GP = 128                     # edges per matmul group
GROUPS = CHUNK // GP         # 4
WSLOTS = 64                  # slot window per chunk
SB = 4                       # chunks per superblock


# ---------------------------------------------------------------- host side

def _preprocess(u, v, e_vals, e_idx_v, e_idx_u):
    u = np.ascontiguousarray(np.asarray(u, np.float32))
    v = np.ascontiguousarray(np.asarray(v, np.float32))
    e_vals = np.ascontiguousarray(np.asarray(e_vals, np.float32))
    e_idx_u = np.asarray(e_idx_u).astype(np.int64)
    e_idx_v = np.asarray(e_idx_v).astype(np.int64)

    perm = np.argsort(e_idx_u, kind="stable")
    su = e_idx_u[perm]
    sv = e_idx_v[perm]
    se = e_vals[perm]

    core_lo = np.searchsorted(su, np.arange(NCORES) * U_PER, side="left")
    core_hi = np.searchsorted(su, (np.arange(NCORES) + 1) * U_PER, side="left")

    cores = []
    for k in range(NCORES):
        lo, hi = int(core_lo[k]), int(core_hi[k])
        su_l = (su[lo:hi] - k * U_PER).astype(np.int64)
        n = hi - lo
        starts, bases = [], []
        i = 0
        while i < n:
            base = int(su_l[i])
            j = min(i + CHUNK, n)
            j = min(j, int(np.searchsorted(su_l, base + WSLOTS, side="left")))
            if j < n:
                # step back to a u-boundary so no u straddles chunks
                j2 = int(np.searchsorted(su_l, su_l[j - 1], side="left"))
                if j2 > i and su_l[j - 1] == su_l[j]:
                    j = j2
            assert j > i, "u degree >= CHUNK unsupported"
            starts.append(i)
            bases.append(base)
            i = j
        starts.append(n)
        nchunks = len(bases)

        col_of_slot = np.full(U_PER, -1, np.int64)
        for c in range(nchunks):
            s0, s1 = starts[c], starts[c + 1]
            slots = np.unique(su_l[s0:s1])
            assert slots.max() - bases[c] < WSLOTS
            col_of_slot[slots] = WSLOTS * c + (slots - bases[c])
        uncovered = np.flatnonzero(col_of_slot < 0)
        cores.append(dict(lo=lo, hi=hi, su_l=su_l, starts=starts, bases=bases,
                          nchunks=nchunks, col_of_slot=col_of_slot,
                          uncovered=uncovered))

    need = max(c["nchunks"] + (len(c["uncovered"]) + WSLOTS - 1) // WSLOTS + 1
               for c in cores)
    B = need + (-need) % SB          # superblocks of SB chunks
    NE = B * CHUNK
    C = B * WSLOTS

    per_core = []
    for k in range(NCORES):
        ci = cores[k]
        lo, hi = ci["lo"], ci["hi"]
        su_l, starts, bases = ci["su_l"], ci["starts"], ci["bases"]
        nchunks = ci["nchunks"]

        col_of_slot = ci["col_of_slot"].copy()
        unc = ci["uncovered"]
        if len(unc):
            cols = WSLOTS * nchunks + np.arange(len(unc))
            assert cols.max() < C
            col_of_slot[unc] = cols
        assert (col_of_slot >= 0).all()

        x_T = np.zeros((128, NE), np.float32)      # [uT ; vT]
        # e4: 4 chunks side by side in rows 0:16 / 32:48 / 64:80 / 96:112
        e4_T = np.zeros((112, NE // SB), np.float32)
        slot_cols = np.full((GP, GROUPS * B), -1.0, np.float32)
        n = hi - lo
        if n:
            x_src = np.empty((128, n), np.float32)
            x_src[0:64] = u[su[lo:hi]].T
            x_src[64:128] = v[sv[lo:hi]].T
            e_src = se[lo:hi].T                     # [16, n]
        for c in range(nchunks):
            s0, s1 = starts[c], starts[c + 1]
            m = s1 - s0
            x_T[:, c * CHUNK:c * CHUNK + m] = x_src[:, s0:s1]
            r0 = 32 * (c % SB)
            col0 = (c // SB) * CHUNK
            e4_T[r0:r0 + 16, col0:col0 + m] = e_src[:, s0:s1]
            full = np.full(CHUNK, -1.0, np.float32)
            full[:m] = (su_l[s0:s1] - bases[c]).astype(np.float32)
            slot_cols[:, GROUPS * c:GROUPS * (c + 1)] = \
                full.reshape(GROUPS, GP).T

        u_T_compact = np.zeros((64, C), np.float32)
        u_T_compact[:, col_of_slot] = u[k * U_PER:(k + 1) * U_PER].T

        per_core.append(dict(x_T=x_T, e4_T=e4_T, slot_cols=slot_cols,
                             u_T_compact=u_T_compact,
                             col_of_slot=col_of_slot))
    return per_core, B, NE, C


# ---------------------------------------------------------------- device side

def _build_program(B, NE, C, io_dtype_np):
    import concourse.bacc as bacc
    import concourse.mybir as mybir
    import concourse.tile as tile

    FB = (C + 511) // 512               # f-MLP chunks
    md = mybir.dt.from_np(np.dtype(io_dtype_np))
    f32 = mybir.dt.float32
    Relu = mybir.ActivationFunctionType.Relu
    Copy = mybir.ActivationFunctionType.Copy
    Alu = mybir.AluOpType

    nc = bacc.Bacc("TRN2", target_bir_lowering=False, debug=False,
                   num_devices=NCORES)

    # I/O
    x_T = nc.dram_tensor("x_T", [128, NE], md, kind="ExternalInput")
    e4_T = nc.dram_tensor("e4_T", [112, NE // SB], md, kind="ExternalInput")
    slot_cols = nc.dram_tensor("slot_cols", [GP, GROUPS * B], md,
                               kind="ExternalInput")
    u_Tc = nc.dram_tensor("u_Tc", [64, C], md, kind="ExternalInput")
    w1a = nc.dram_tensor("w1a", [128, 128], md, kind="ExternalInput")
    w1e4 = nc.dram_tensor("w1e4", [112, 128], md, kind="ExternalInput")
    w2 = nc.dram_tensor("w2", [128, 64], md, kind="ExternalInput")
    fw1 = nc.dram_tensor("fw1", [128, 128], md, kind="ExternalInput")
    fw2 = nc.dram_tensor("fw2", [128, 64], md, kind="ExternalInput")
    b1 = nc.dram_tensor("b1", [128, 1], f32, kind="ExternalInput")
    b2m = nc.dram_tensor("b2m", [GP, 64 * GROUPS], f32, kind="ExternalInput")
    fb1 = nc.dram_tensor("fb1", [128, 1], f32, kind="ExternalInput")
    fb2 = nc.dram_tensor("fb2", [64, 1], f32, kind="ExternalInput")
    iota16 = nc.dram_tensor("iota16", [GP, 64 * GROUPS * SB], md,
                            kind="ExternalInput")
    out_T = nc.dram_tensor("out_T", [64, C], f32, kind="ExternalOutput")

    NSB = B // SB                       # superblocks

    with tile.TileContext(nc) as tc:
        with (
            tc.tile_pool(name="consts", bufs=1) as cp,
            tc.tile_pool(name="xf", bufs=1) as xfp,
            tc.tile_pool(name="xin", bufs=3) as xp,
            tc.tile_pool(name="work", bufs=4) as wp,
            tc.tile_pool(name="ps1", bufs=2, space="PSUM") as p1,
            tc.tile_pool(name="ps2", bufs=2, space="PSUM") as p2,
            tc.tile_pool(name="ps3", bufs=1, space="PSUM") as p3,
            tc.tile_pool(name="psf", bufs=1, space="PSUM") as pf,
        ):
            # resident constants
            w1a_s = cp.tile([128, 128], md)
            w1e4_s = cp.tile([112, 128], md)
            w2_s = cp.tile([128, 64], md)
            fw1_s = cp.tile([128, 128], md)
            fw2_s = cp.tile([128, 64], md)
            b1_s = cp.tile([128, 1], f32)
            b2m_s = cp.tile([GP, 64 * GROUPS], f32)
            fb1_s = cp.tile([128, 1], f32)
            fb2_s = cp.tile([64, 1], f32)
            iota_s = cp.tile([GP, 64 * GROUPS * SB], md)
            slot_s = cp.tile([GP, GROUPS * B], md)
            for dst, src in [(w1a_s, w1a), (w1e4_s, w1e4), (w2_s, w2),
                             (fw1_s, fw1), (fw2_s, fw2), (b1_s, b1),
                             (b2m_s, b2m), (fb1_s, fb1), (fb2_s, fb2),
                             (iota_s, iota16), (slot_s, slot_cols)]:
                nc.sync.dma_start(dst[:], src[:])

            # xf: rows 0:64 aggT (flushed per superblock), rows 64:128 uT
            xf = xfp.tile([128, C], md)
            nc.sync.dma_start(xf[64:128, :], u_Tc[:])

            # f-MLP chunk emitter (interleaved into the main loop)
            f_done = [0]

            def emit_f(fc):
                w = min(512, C - 512 * fc)
                fsl = slice(512 * fc, 512 * fc + w)
                zf1 = pf.tile([128, 512], f32, tag="zf")
                nc.tensor.matmul(zf1[:, :w], lhsT=fw1_s[:], rhs=xf[:, fsl],
                                 start=True, stop=True)
                hf = wp.tile([128, 512], md, tag="hf")
                nc.scalar.activation(hf[:, :w], zf1[:, :w], Relu,
                                     bias=fb1_s[:])
                zf2 = pf.tile([64, 512], f32, tag="zf")
                nc.tensor.matmul(zf2[:, :w], lhsT=fw2_s[:], rhs=hf[:, :w],
                                 start=True, stop=True)
                ot = wp.tile([64, 512], f32, tag="ot")
                nc.scalar.activation(ot[:, :w], zf2[:, :w], Relu,
                                     bias=fb2_s[:])
                nc.sync.dma_start(out_T[:, fsl], ot[:, :w])
                f_done[0] = fc + 1

            def make_oh(m):
                oh = wp.tile([GP, 64 * GROUPS * SB], md, tag="oh", name="oh")
                nc.vector.tensor_tensor(
                    oh[:].rearrange("p (g s) -> p g s", g=GROUPS * SB),
                    iota_s[:].rearrange("p (g s) -> p g s", g=GROUPS * SB),
                    slot_s[:, GROUPS * SB * m:GROUPS * SB * (m + 1)]
                        [:, :, None]
                        .to_broadcast([GP, GROUPS * SB, WSLOTS]),
                    op=Alu.is_equal)
                return oh

            oh_next = make_oh(0)

            for m in range(NSB):                    # superblock = 4 chunks
                if m % 2 == 0:                      # 2-superblock x1 batch
                    nb = min(2, NSB - m)
                    x1 = xp.tile([128, 2 * SB * CHUNK], md, tag="x1")
                    nc.sync.dma_start(
                        x1[:, :nb * SB * CHUNK],
                        x_T[:, m * SB * CHUNK:(m + nb) * SB * CHUNK])
                x1off = (m % 2) * SB * CHUNK
                e4 = xp.tile([112, CHUNK], md, tag="e4")
                nc.sync.dma_start(e4[:], e4_T[:, m * CHUNK:(m + 1) * CHUNK])

                oh = oh_next

                # L1: allocate z1 in 2-chunk pairs (one 2-bank PSUM tile) so
                # both chunks' K=16 e matmuls become ready together, pop
                # adjacently, and run concurrently in disjoint PE row groups.
                zp = [p1.tile([128, 2 * CHUNK], f32, tag="z1", name="z1")
                      for _ in range(SB // 2)]
                z1s = [zp[0][:, 0:CHUNK], zp[0][:, CHUNK:2 * CHUNK],
                       zp[1][:, 0:CHUNK], zp[1][:, CHUNK:2 * CHUNK]]
                for q2 in range(SB // 2):
                    for q in (2 * q2, 2 * q2 + 1):
                        r0 = 32 * q
                        nc.tensor.matmul(z1s[q][:],
                                         lhsT=w1e4_s[r0:r0 + 16, :],
                                         rhs=e4[r0:r0 + 16, :],
                                         start=True, stop=False,
                                         tile_position=(r0, 0),
                                         skip_group_check=True)
                    for q in (2 * q2, 2 * q2 + 1):
                        nc.tensor.matmul(z1s[q][:], lhsT=w1a_s[:],
                                         rhs=x1[:, x1off + q * CHUNK:
                                                x1off + (q + 1) * CHUNK],
                                         start=False, stop=True,
                                         skip_group_check=True)

                # dual scatter accumulators: rows 0:64 take groups 0,1 and
                # rows 64:128 take groups 2,3 (distinct PE column groups ->
                # the per-chunk scatter matmuls run in concurrent pairs)
                pT = p3.tile([128, SB * WSLOTS], f32, tag="pT")
                for q in range(SB):                 # per-chunk tail
                    h1 = wp.tile([128, CHUNK], md, tag="h1")
                    nc.scalar.activation(h1[:], z1s[q][:], Relu, bias=b1_s[:])

                    # L2 in column-split halves: edges 0:64 of each group to
                    # PE col groups {0,1}, edges 64:128 to {2,3} (concurrent)
                    z2 = p2.tile([GP, 64 * GROUPS], f32, tag="z2")
                    for g in range(GROUPS):
                        nc.tensor.matmul(z2[0:64, 64 * g:64 * (g + 1)],
                                         lhsT=h1[:, GP * g:GP * g + 64],
                                         rhs=w2_s[:], start=True, stop=True,
                                         tile_position=(0, 0),
                                         skip_group_check=True)
                        nc.tensor.matmul(z2[64:128, 64 * g:64 * (g + 1)],
                                         lhsT=h1[:, GP * g + 64:
                                                GP * (g + 1)],
                                         rhs=w2_s[:], start=True, stop=True,
                                         tile_position=(0, 64),
                                         skip_group_check=True)
                    h2T = wp.tile([GP, 64 * GROUPS], md, tag="h2T")
                    nc.vector.tensor_tensor(h2T[:], z2[:], b2m_s[:],
                                            op=Alu.add)
                    nc.vector.tensor_scalar_max(h2T[:], h2T[:], 0.0)

                    for gh in range(2):             # pairs (0,2) and (1,3)
                        g0, g1 = gh, gh + 2
                        nc.tensor.matmul(pT[0:64, 64 * q:64 * (q + 1)],
                                         lhsT=h2T[:, 64 * g0:64 * (g0 + 1)],
                                         rhs=oh[:, 256 * q + 64 * g0:
                                                256 * q + 64 * (g0 + 1)],
                                         start=(gh == 0), stop=(gh == 1),
                                         tile_position=(0, 0),
                                         skip_group_check=True)
                        nc.tensor.matmul(pT[64:128, 64 * q:64 * (q + 1)],
                                         lhsT=h2T[:, 64 * g1:64 * (g1 + 1)],
                                         rhs=oh[:, 256 * q + 64 * g1:
                                                256 * q + 64 * (g1 + 1)],
                                         start=(gh == 0), stop=(gh == 1),
                                         tile_position=(0, 64),
                                         skip_group_check=True)
                # flush: xf = pT_low + pT_high (DVE reads at most one PSUM
                # operand per op, so stage the low half through ACT first)
                ptmp = wp.tile([64, SB * WSLOTS], f32, tag="ptmp")
                nc.scalar.activation(ptmp[:], pT[0:64, :], Copy)
                nc.vector.tensor_tensor(
                    xf[0:64, SB * WSLOTS * m:SB * WSLOTS * (m + 1)],
                    ptmp[:], pT[64:128, :], op=Alu.add)
                if m + 1 < NSB:                     # prefetch next one-hot
                    oh_next = make_oh(m + 1)
                while (f_done[0] + 1) * 512 <= (m + 1) * SB * WSLOTS:
                    emit_f(f_done[0])

            for fc in range(f_done[0], FB):
                emit_f(fc)

    nc.compile()
    return nc


def _make_in_maps(per_core, consts, io_dtype_np):
    in_maps = []
    for pc in per_core:
        m = dict(
            x_T=pc["x_T"].astype(io_dtype_np),
            e4_T=pc["e4_T"].astype(io_dtype_np),
            slot_cols=pc["slot_cols"].astype(io_dtype_np),
            u_Tc=pc["u_T_compact"].astype(io_dtype_np),
            **{k: v for k, v in consts.items()},
        )
        in_maps.append(m)
    return in_maps


def _make_consts(g_w1, g_b1, g_w2, g_b2, f_w1, f_b1, f_w2, f_b2, io_dtype_np):
    dt = io_dtype_np
    g_w1 = np.asarray(g_w1, np.float32)
    w1e4 = np.zeros((112, 128), np.float32)
    for q in range(SB):
        w1e4[32 * q:32 * q + 16] = g_w1[128:144]
    # f-MLP input is [aggT ; uT] (agg rows first), so permute f_w1 rows
    f_w1 = np.asarray(f_w1, np.float32)
    f_w1p = np.concatenate([f_w1[64:128], f_w1[0:64]], axis=0)
    return dict(
        w1a=np.ascontiguousarray(g_w1[0:128]).astype(dt),
        w1e4=w1e4.astype(dt),
        w2=np.asarray(g_w2, np.float32).astype(dt),
        fw1=np.ascontiguousarray(f_w1p).astype(dt),
        fw2=np.asarray(f_w2, np.float32).astype(dt),
        b1=np.asarray(g_b1, np.float32).reshape(128, 1),
        b2m=np.ascontiguousarray(
            np.tile(np.asarray(g_b2, np.float32)[None, :], (GP, GROUPS))),
        fb1=np.asarray(f_b1, np.float32).reshape(128, 1),
        fb2=np.asarray(f_b2, np.float32).reshape(64, 1),
        iota16=np.ascontiguousarray(
            np.tile(np.arange(WSLOTS, dtype=np.float32)[None, :],
                    (GP, GROUPS * SB))).astype(dt),
    )


_last_run_info = {}


def kernel(u, v, e_vals, e_idx_v, e_idx_u, g_w1, g_b1, g_w2, g_b2,
           f_w1, f_b1, f_w2, f_b2, _trace=False):
    import ml_dtypes
    from concourse import bass_utils

    io_dtype_np = ml_dtypes.bfloat16

    per_core, B, NE, C = _preprocess(u, v, e_vals, e_idx_v, e_idx_u)
    consts = _make_consts(g_w1, g_b1, g_w2, g_b2, f_w1, f_b1, f_w2, f_b2,
                          io_dtype_np)
    nc = _build_program(B, NE, C, io_dtype_np)
    in_maps = _make_in_maps(per_core, consts, io_dtype_np)

    res = bass_utils.run_bass_kernel_spmd(
        nc, in_maps, core_ids=list(range(NCORES)), trace=_trace)
    _last_run_info.clear()
    _last_run_info.update(B=B, NE=NE, C=C, res=res)

    out = np.zeros((U, 64), np.float32)
    for k in range(NCORES):
        out_T = res.results[k]["out_T"]
        cols = per_core[k]["col_of_slot"]
        out[k * U_PER:(k + 1) * U_PER] = out_T[:, cols].T
    return out


# revision 20
# speedup vs baseline: 1.6546x; 1.1585x over previous
"""Trainium2 Bass kernel for nn_HalfConv_876173328516 (GNN message passing).

Strategy
--------
Host: sort edges by e_idx_u; core k owns u rows [k*6250, (k+1)*6250), so the 8
cores are fully independent (no collectives). Per-edge inputs are expanded and
transposed on the host into a [128, NE] u||v feature stream plus a packed
[112, NE/4] e_vals stream per core (4 chunks side by side in PE row groups
0-15/32-47/64-79/96-111), with edges packed into 512-edge chunks that
(a) never split one u across chunks and (b) span < 64 u-slots.

Device (per core, per superblock of 4 chunks = 2048 edges):
  L1   z1[q] = W1uv.T @ x_q       (4 matmuls, one w1a weight-load context)
       z1[q] += W1e.T @ e_q       (4 K=16 matmuls packed in disjoint PE row
                                   groups -> run concurrently)
       h1 = relu(z1 + b1)         (ACT, -> SBUF bf16)
  L2   per 128-edge group: z2 = h1_g.T @ W2    (edge-major output)
       h2 = relu(z2 + b2)                      (DVE add + max)
  SUM  one-hot[e, slot] built once per superblock ([128, 1024] is_equal)
       pT[64 feats, 64 slots/chunk] += h2_g.T @ oh_g  (PSUM accumulate)
       flush pT -> xf[0:64, 256*m4:...]        (one DVE copy per superblock)
  f-MLP over all compact slot columns: xf = [aggT ; uT], two matmuls + relus.

Host: out[u] = out_T[:, col_of_slot[u]].T per core.
"""

import numpy as np

U, V, E = 50000, 50000, 800000
NCORES = 8
U_PER = U // NCORES          # 6250
CHUNK = 512                  # edges per chunk
GP = 128                     # edges per matmul group
GROUPS = CHUNK // GP         # 4
WSLOTS = 64                  # slot window per chunk
SB = 4                       # chunks per superblock


# ---------------------------------------------------------------- host side

def _preprocess(u, v, e_vals, e_idx_v, e_idx_u):
    u = np.ascontiguousarray(np.asarray(u, np.float32))
    v = np.ascontiguousarray(np.asarray(v, np.float32))
    e_vals = np.ascontiguousarray(np.asarray(e_vals, np.float32))
    e_idx_u = np.asarray(e_idx_u).astype(np.int64)
    e_idx_v = np.asarray(e_idx_v).astype(np.int64)

    perm = np.argsort(e_idx_u, kind="stable")
    su = e_idx_u[perm]
    sv = e_idx_v[perm]
    se = e_vals[perm]

    core_lo = np.searchsorted(su, np.arange(NCORES) * U_PER, side="left")
    core_hi = np.searchsorted(su, (np.arange(NCORES) + 1) * U_PER, side="left")

    cores = []
    for k in range(NCORES):
        lo, hi = int(core_lo[k]), int(core_hi[k])
        su_l = (su[lo:hi] - k * U_PER).astype(np.int64)
        n = hi - lo
        starts, bases = [], []
        i = 0
        while i < n:
            base = int(su_l[i])
            j = min(i + CHUNK, n)
            j = min(j, int(np.searchsorted(su_l, base + WSLOTS, side="left")))
            if j < n:
                # step back to a u-boundary so no u straddles chunks
                j2 = int(np.searchsorted(su_l, su_l[j - 1], side="left"))
                if j2 > i and su_l[j - 1] == su_l[j]:
                    j = j2
            assert j > i, "u degree >= CHUNK unsupported"
            starts.append(i)
            bases.append(base)
            i = j
        starts.append(n)
        nchunks = len(bases)

        col_of_slot = np.full(U_PER, -1, np.int64)
        for c in range(nchunks):
            s0, s1 = starts[c], starts[c + 1]
            slots = np.unique(su_l[s0:s1])
            assert slots.max() - bases[c] < WSLOTS
            col_of_slot[slots] = WSLOTS * c + (slots - bases[c])
        uncovered = np.flatnonzero(col_of_slot < 0)
        cores.append(dict(lo=lo, hi=hi, su_l=su_l, starts=starts, bases=bases,
                          nchunks=nchunks, col_of_slot=col_of_slot,
                          uncovered=uncovered))

    need = max(c["nchunks"] + (len(c["uncovered"]) + WSLOTS - 1) // WSLOTS + 1
               for c in cores)
    B = need + (-need) % SB          # superblocks of SB chunks
    NE = B * CHUNK
    C = B * WSLOTS

    per_core = []
    for k in range(NCORES):
        ci = cores[k]
        lo, hi = ci["lo"], ci["hi"]
        su_l, starts, bases = ci["su_l"], ci["starts"], ci["bases"]
        nchunks = ci["nchunks"]

        col_of_slot = ci["col_of_slot"].copy()
        unc = ci["uncovered"]
        if len(unc):
            cols = WSLOTS * nchunks + np.arange(len(unc))
            assert cols.max() < C
            col_of_slot[unc] = cols
        assert (col_of_slot >= 0).all()

        x_T = np.zeros((128, NE), np.float32)      # [uT ; vT]
        # e4: 4 chunks side by side in rows 0:16 / 32:48 / 64:80 / 96:112
        e4_T = np.zeros((112, NE // SB), np.float32)
        slot_cols = np.full((GP, GROUPS * B), -1.0, np.float32)
        n = hi - lo
        if n:
            x_src = np.empty((128, n), np.float32)
            x_src[0:64] = u[su[lo:hi]].T
            x_src[64:128] = v[sv[lo:hi]].T
            e_src = se[lo:hi].T                     # [16, n]
        for c in range(nchunks):
            s0, s1 = starts[c], starts[c + 1]
            m = s1 - s0
            x_T[:, c * CHUNK:c * CHUNK + m] = x_src[:, s0:s1]
            r0 = 32 * (c % SB)
            col0 = (c // SB) * CHUNK
            e4_T[r0:r0 + 16, col0:col0 + m] = e_src[:, s0:s1]
            full = np.full(CHUNK, -1.0, np.float32)
            full[:m] = (su_l[s0:s1] - bases[c]).astype(np.float32)
            slot_cols[:, GROUPS * c:GROUPS * (c + 1)] = \
                full.reshape(GROUPS, GP).T

        u_T_compact = np.zeros((64, C), np.float32)
        u_T_compact[:, col_of_slot] = u[k * U_PER:(k + 1) * U_PER].T

        oh_h = (slot_cols[:, :, None] ==
                np.arange(WSLOTS, dtype=np.float32)).astype(np.float32)
        oh_h = oh_h.reshape(GP, GROUPS * B * WSLOTS)
        per_core.append(dict(x_T=x_T, e4_T=e4_T, oh_T=oh_h,
                             u_T_compact=u_T_compact,
                             col_of_slot=col_of_slot))
    return per_core, B, NE, C


# ---------------------------------------------------------------- device side

def _build_program(B, NE, C, io_dtype_np):
    import concourse.bacc as bacc
    import concourse.mybir as mybir
    import concourse.tile as tile

    FB = (C + 511) // 512               # f-MLP chunks
    md = mybir.dt.from_np(np.dtype(io_dtype_np))
    f32 = mybir.dt.float32
    Relu = mybir.ActivationFunctionType.Relu
    Copy = mybir.ActivationFunctionType.Copy
    Alu = mybir.AluOpType

    nc = bacc.Bacc("TRN2", target_bir_lowering=False, debug=False,
                   num_devices=NCORES)

    # I/O
    x_T = nc.dram_tensor("x_T", [128, NE], md, kind="ExternalInput")
    e4_T = nc.dram_tensor("e4_T", [112, NE // SB], md, kind="ExternalInput")
    u_Tc = nc.dram_tensor("u_Tc", [64, C], md, kind="ExternalInput")
    w1a = nc.dram_tensor("w1a", [128, 128], md, kind="ExternalInput")
    w1e4 = nc.dram_tensor("w1e4", [112, 128], md, kind="ExternalInput")
    w2 = nc.dram_tensor("w2", [128, 64], md, kind="ExternalInput")
    fw1 = nc.dram_tensor("fw1", [128, 128], md, kind="ExternalInput")
    fw2 = nc.dram_tensor("fw2", [128, 64], md, kind="ExternalInput")
    b1 = nc.dram_tensor("b1", [128, 1], f32, kind="ExternalInput")
    b2m = nc.dram_tensor("b2m", [GP, 2 * 64 * GROUPS], f32,
                         kind="ExternalInput")
    fb1 = nc.dram_tensor("fb1", [128, 1], f32, kind="ExternalInput")
    fb2 = nc.dram_tensor("fb2", [64, 1], f32, kind="ExternalInput")
    oh_T = nc.dram_tensor("oh_T", [GP, 64 * GROUPS * B], md,
                          kind="ExternalInput")
    out_T = nc.dram_tensor("out_T", [64, C], f32, kind="ExternalOutput")

    NSB = B // SB                       # superblocks

    with tile.TileContext(nc) as tc:
        with (
            tc.tile_pool(name="consts", bufs=1) as cp,
            tc.tile_pool(name="xf", bufs=1) as xfp,
            tc.tile_pool(name="xin", bufs=3) as xp,
            tc.tile_pool(name="work", bufs=4) as wp,
            tc.tile_pool(name="ps1", bufs=2, space="PSUM") as p1,
            tc.tile_pool(name="ps2", bufs=2, space="PSUM") as p2,
            tc.tile_pool(name="ps3", bufs=1, space="PSUM") as p3,
            tc.tile_pool(name="psf", bufs=1, space="PSUM") as pf,
        ):
            # resident constants
            w1a_s = cp.tile([128, 128], md)
            w1e4_s = cp.tile([112, 128], md)
            w2_s = cp.tile([128, 64], md)
            fw1_s = cp.tile([128, 128], md)
            fw2_s = cp.tile([128, 64], md)
            b1_s = cp.tile([128, 1], f32)
            b2m_s = cp.tile([GP, 2 * 64 * GROUPS], f32)
            fb1_s = cp.tile([128, 1], f32)
            fb2_s = cp.tile([64, 1], f32)
            for dst, src in [(w1a_s, w1a), (w1e4_s, w1e4), (w2_s, w2),
                             (fw1_s, fw1), (fw2_s, fw2), (b1_s, b1),
                             (b2m_s, b2m), (fb1_s, fb1), (fb2_s, fb2)]:
                nc.sync.dma_start(dst[:], src[:])

            # xf: rows 0:64 aggT (flushed per superblock), rows 64:128 uT
            xf = xfp.tile([128, C], md)
            nc.sync.dma_start(xf[64:128, :], u_Tc[:])

            # f-MLP chunk emitter (interleaved into the main loop)
            f_done = [0]

            def emit_f(fc):
                w = min(512, C - 512 * fc)
                fsl = slice(512 * fc, 512 * fc + w)
                zf1 = pf.tile([128, 512], f32, tag="zf")
                nc.tensor.matmul(zf1[:, :w], lhsT=fw1_s[:], rhs=xf[:, fsl],
                                 start=True, stop=True)
                hf = wp.tile([128, 512], md, tag="hf")
                nc.scalar.activation(hf[:, :w], zf1[:, :w], Relu,
                                     bias=fb1_s[:])
                zf2 = pf.tile([64, 512], f32, tag="zf")
                nc.tensor.matmul(zf2[:, :w], lhsT=fw2_s[:], rhs=hf[:, :w],
                                 start=True, stop=True)
                ot = wp.tile([64, 512], f32, tag="ot")
                nc.scalar.activation(ot[:, :w], zf2[:, :w], Relu,
                                     bias=fb2_s[:])
                nc.sync.dma_start(out_T[:, fsl], ot[:, :w])
                f_done[0] = fc + 1

            for m in range(NSB):                    # superblock = 4 chunks
                if m % 2 == 0:                      # 2-superblock x1 batch
                    nb = min(2, NSB - m)
                    x1 = xp.tile([128, 2 * SB * CHUNK], md, tag="x1")
                    nc.sync.dma_start(
                        x1[:, :nb * SB * CHUNK],
                        x_T[:, m * SB * CHUNK:(m + nb) * SB * CHUNK])
                x1off = (m % 2) * SB * CHUNK
                e4 = xp.tile([112, CHUNK], md, tag="e4")
                nc.sync.dma_start(e4[:], e4_T[:, m * CHUNK:(m + 1) * CHUNK])
                oh = xp.tile([GP, 64 * GROUPS * SB], md, tag="oh")
                nc.sync.dma_start(
                    oh[:], oh_T[:, 64 * GROUPS * SB * m:
                                64 * GROUPS * SB * (m + 1)])

                # L1: allocate z1 in 2-chunk pairs (one 2-bank PSUM tile) so
                # both chunks' K=16 e matmuls become ready together, pop
                # adjacently, and run concurrently in disjoint PE row groups.
                zp = [p1.tile([128, 2 * CHUNK], f32, tag="z1", name="z1")
                      for _ in range(SB // 2)]
                z1s = [zp[0][:, 0:CHUNK], zp[0][:, CHUNK:2 * CHUNK],
                       zp[1][:, 0:CHUNK], zp[1][:, CHUNK:2 * CHUNK]]
                for q2 in range(SB // 2):
                    for q in (2 * q2, 2 * q2 + 1):
                        r0 = 32 * q
                        nc.tensor.matmul(z1s[q][:],
                                         lhsT=w1e4_s[r0:r0 + 16, :],
                                         rhs=e4[r0:r0 + 16, :],
                                         start=True, stop=False,
                                         tile_position=(r0, 0),
                                         skip_group_check=True)
                    for q in (2 * q2, 2 * q2 + 1):
                        nc.tensor.matmul(z1s[q][:], lhsT=w1a_s[:],
                                         rhs=x1[:, x1off + q * CHUNK:
                                                x1off + (q + 1) * CHUNK],
                                         start=False, stop=True,
                                         skip_group_check=True)

                # dual scatter accumulators: rows 0:64 take groups 0,1 and
                # rows 64:128 take groups 2,3 (distinct PE column groups ->
                # the per-chunk scatter matmuls run in concurrent pairs)
                pT = p3.tile([128, SB * WSLOTS], f32, tag="pT")
                for q2 in range(SB // 2):           # per-pair tail
                    h1p = wp.tile([128, 2 * CHUNK], md, tag="h1")
                    nc.scalar.activation(h1p[:], zp[q2][:], Relu,
                                         bias=b1_s[:])

                    # L2 column-split halves; both chunks of the pair share
                    # one single-bank z2 tile and one ADD+MAX pass
                    z2p = p2.tile([GP, 2 * 64 * GROUPS], f32, tag="z2")
                    for qq in range(2):
                        h1o = qq * CHUNK
                        z2o = qq * 64 * GROUPS
                        for g in range(GROUPS):
                            nc.tensor.matmul(
                                z2p[0:64, z2o + 64 * g:z2o + 64 * (g + 1)],
                                lhsT=h1p[:, h1o + GP * g:h1o + GP * g + 64],
                                rhs=w2_s[:], start=True, stop=True,
                                tile_position=(0, 0),
                                skip_group_check=True)
                            nc.tensor.matmul(
                                z2p[64:128, z2o + 64 * g:z2o + 64 * (g + 1)],
                                lhsT=h1p[:, h1o + GP * g + 64:
                                         h1o + GP * (g + 1)],
                                rhs=w2_s[:], start=True, stop=True,
                                tile_position=(0, 64),
                                skip_group_check=True)
                    h2Tp = wp.tile([GP, 2 * 64 * GROUPS], md, tag="h2T")
                    nc.vector.tensor_tensor(h2Tp[:], z2p[:], b2m_s[:],
                                            op=Alu.add)
                    nc.vector.tensor_scalar_max(h2Tp[:], h2Tp[:], 0.0)

                    for qq in range(2):
                        q = 2 * q2 + qq
                        ho = qq * 64 * GROUPS
                        for gh in range(2):         # pairs (0,2) and (1,3)
                            g0, g1 = gh, gh + 2
                            nc.tensor.matmul(
                                pT[0:64, 64 * q:64 * (q + 1)],
                                lhsT=h2Tp[:, ho + 64 * g0:ho + 64 * (g0 + 1)],
                                rhs=oh[:, 256 * q + 64 * g0:
                                       256 * q + 64 * (g0 + 1)],
                                start=(gh == 0), stop=(gh == 1),
                                tile_position=(0, 0),
                                skip_group_check=True)
                            nc.tensor.matmul(
                                pT[64:128, 64 * q:64 * (q + 1)],
                                lhsT=h2Tp[:, ho + 64 * g1:ho + 64 * (g1 + 1)],
                                rhs=oh[:, 256 * q + 64 * g1:
                                       256 * q + 64 * (g1 + 1)],
                                start=(gh == 0), stop=(gh == 1),
                                tile_position=(0, 64),
                                skip_group_check=True)
                # flush: xf = pT_low + pT_high (DVE reads at most one PSUM
                # operand per op, so stage the low half through ACT first)
                ptmp = wp.tile([64, SB * WSLOTS], f32, tag="ptmp")
                nc.scalar.activation(ptmp[:], pT[0:64, :], Copy)
                nc.vector.tensor_tensor(
                    xf[0:64, SB * WSLOTS * m:SB * WSLOTS * (m + 1)],
                    ptmp[:], pT[64:128, :], op=Alu.add)
                while (f_done[0] + 1) * 512 <= (m + 1) * SB * WSLOTS:
                    emit_f(f_done[0])

            for fc in range(f_done[0], FB):
                emit_f(fc)

    nc.compile()
    return nc


def _make_in_maps(per_core, consts, io_dtype_np):
    in_maps = []
    for pc in per_core:
        m = dict(
            x_T=pc["x_T"].astype(io_dtype_np),
            e4_T=pc["e4_T"].astype(io_dtype_np),
            oh_T=pc["oh_T"].astype(io_dtype_np),
            u_Tc=pc["u_T_compact"].astype(io_dtype_np),
            **{k: v for k, v in consts.items()},
        )
        in_maps.append(m)
    return in_maps


def _make_consts(g_w1, g_b1, g_w2, g_b2, f_w1, f_b1, f_w2, f_b2, io_dtype_np):
    dt = io_dtype_np
    g_w1 = np.asarray(g_w1, np.float32)
    w1e4 = np.zeros((112, 128), np.float32)
    for q in range(SB):
        w1e4[32 * q:32 * q + 16] = g_w1[128:144]
    # f-MLP input is [aggT ; uT] (agg rows first), so permute f_w1 rows
    f_w1 = np.asarray(f_w1, np.float32)
    f_w1p = np.concatenate([f_w1[64:128], f_w1[0:64]], axis=0)
    return dict(
        w1a=np.ascontiguousarray(g_w1[0:128]).astype(dt),
        w1e4=w1e4.astype(dt),
        w2=np.asarray(g_w2, np.float32).astype(dt),
        fw1=np.ascontiguousarray(f_w1p).astype(dt),
        fw2=np.asarray(f_w2, np.float32).astype(dt),
        b1=np.asarray(g_b1, np.float32).reshape(128, 1),
        b2m=np.ascontiguousarray(
            np.tile(np.asarray(g_b2, np.float32)[None, :],
                    (GP, 2 * GROUPS))),
        fb1=np.asarray(f_b1, np.float32).reshape(128, 1),
        fb2=np.asarray(f_b2, np.float32).reshape(64, 1),
    )


_last_run_info = {}


def kernel(u, v, e_vals, e_idx_v, e_idx_u, g_w1, g_b1, g_w2, g_b2,
           f_w1, f_b1, f_w2, f_b2, _trace=False):
    import ml_dtypes
    from concourse import bass_utils

    io_dtype_np = ml_dtypes.bfloat16

    per_core, B, NE, C = _preprocess(u, v, e_vals, e_idx_v, e_idx_u)
    consts = _make_consts(g_w1, g_b1, g_w2, g_b2, f_w1, f_b1, f_w2, f_b2,
                          io_dtype_np)
    nc = _build_program(B, NE, C, io_dtype_np)
    in_maps = _make_in_maps(per_core, consts, io_dtype_np)

    res = bass_utils.run_bass_kernel_spmd(
        nc, in_maps, core_ids=list(range(NCORES)), trace=_trace)
    _last_run_info.clear()
    _last_run_info.update(B=B, NE=NE, C=C, res=res)

    out = np.zeros((U, 64), np.float32)
    for k in range(NCORES):
        out_T = res.results[k]["out_T"]
        cols = per_core[k]["col_of_slot"]
        out[k * U_PER:(k + 1) * U_PER] = out_T[:, cols].T
    return out
